# revision 2
# baseline (speedup 1.0000x reference)
"""Performer (FAVOR+) linear attention kernel for Trainium2, 8 NeuronCores.

Problem (hardcoded): B=8, L=2048, D=M=256, fp32.
  phi(X)[b,l,m] = exp(X[b,l]@proj[m] - 0.5*||X[:,l,:]||_F) / sqrt(M)
  S = phiK^T V (per batch), z = sum_l phiK, out = (phiQ@S) / (phiQ.z)

Sharding: data-parallel over batch, one batch per core. The per-timestep
Frobenius norm couples all batches, so each core computes a partial
sum-of-squares over its K slice and an 8-core AllReduce (8KB) produces the
global norm. phiQ's per-l scale and all 1/sqrt(M) factors cancel in num/den
and are skipped.

Matmuls run in float32r (fp32 bits, 1 PE cycle/moving-col vs 4; measured
~2.3e-4 max rel err on HW). The fp32r ISA requires even moving/dst free
sizes, so V is padded host-side to [l, V|1|0] (width 258) which also fuses
the S and z matmuls into one.

DMA discipline: every descriptor costs ~0.6us on the shared HWDGE generator
and stalls the issuing engine's sequencer, so the kernel uses few, large
DMAs, all on the SP (sync) queue - never on the Activation engine, which
carries the serial exp() work. K loads first (it feeds the AllReduce
chain), then Q, then the AllReduce bounce (slotted where the queue is
naturally idle), then V; output is staged in SBUF and stored in 4 big
chunks. -0.5*sqrt(ss) is computed with a DVE Newton-rsqrt so ACT stays
pure-Exp (single activation-table load).
"""

import os
import numpy as np

B = 8
L = 2048
D = 256
P = 128
LT = L // P     # 16 l-tiles of 128
DT = D // P     # 2 d-stripes of 128
MT = D // P     # 2 m-stripes of 128
NQ = 512        # moving free-size for the phiQ matmuls
CP = D + 2      # V | ones | zero-pad; even width required by fp32r matmul
NC = 2          # V chunks
LC = LT // NC   # 8 l-tiles per V chunk
SG = 2          # l-tiles per output store

_CACHE = {}


def _build(_mock_collective=False):
    from concourse import bass, bacc, tile

    mybir = bass.mybir
    f32 = mybir.dt.float32
    f32r = mybir.dt.float32r
    bf16 = mybir.dt.bfloat16
    AF = mybir.ActivationFunctionType

    nc = bacc.Bacc("TRN2", target_bir_lowering=False, debug=False, num_devices=B)

    KT = nc.declare_dram_parameter("KT", [D, L], bf16, isOutput=False)
    QT = nc.declare_dram_parameter("QT", [D, L], bf16, isOutput=False)
    Vn = nc.declare_dram_parameter("V", [L, CP], f32r, isOutput=False)
    PT = nc.declare_dram_parameter("PT", [D, D], bf16, isOutput=False)
    OUT = nc.declare_dram_parameter("OUT", [L, D], f32, isOutput=True)

    with tile.TileContext(nc) as tc:
        with (
            tc.tile_pool(name="cst", bufs=1) as cst,
            tc.tile_pool(name="psum", bufs=2, space="PSUM") as psum,
            tc.tile_pool(name="psums", bufs=1, space="PSUM") as psums,
            tc.tile_pool(name="dram", bufs=2, space="DRAM") as dram,
        ):
            pt = [cst.tile([P, D], bf16, tag=f"pt{i}", name=f"pt{i}")
                  for i in range(DT)]
            kt = [cst.tile([P, L], bf16, tag=f"kt{i}", name=f"kt{i}")
                  for i in range(DT)]
            qt = [cst.tile([P, L], bf16, tag=f"qt{i}", name=f"qt{i}")
                  for i in range(DT)]
            vall = [cst.tile([P, LC * CP], f32r, tag=f"vall{c}", name=f"vall{c}")
                    for c in range(NC)]
            sq = [cst.tile([P, L], f32, tag=f"sq{i}", name=f"sq{i}")
                  for i in range(DT)]
            ones = cst.tile([P, 1], f32, tag="ones")
            ssb = cst.tile([P, LT], f32, tag="ssb")
            sstot = cst.tile([P, LT], f32, tag="sstot")
            nrm = cst.tile([P, LT], f32, tag="nrm")
            biasn = cst.tile([P, LT], f32, tag="biasn")
            eq = [cst.tile([P, L], f32r, tag=f"eq{i}", name=f"eq{i}")
                  for i in range(MT)]
            ek = cst.tile([P, LT * D], f32r, tag="ek")
            obig = cst.tile([P, LT * D], f32, tag="obig")

            # ---- input loads, all on the SP queue: K first (it feeds the
            # AllReduce chain), split in 4 so squaring starts early, then
            # proj and Q; the AllReduce bounce and V are queued later ----
            H = L // 2
            for h in range(2):
                for i in range(DT):
                    nc.sync.dma_start(
                        out=kt[i][:, h * H:(h + 1) * H],
                        in_=KT[i * P:(i + 1) * P, h * H:(h + 1) * H],
                    )
            for i in range(DT):
                nc.sync.dma_start(out=pt[i][:], in_=PT[i * P:(i + 1) * P, :])
            for h in range(2):
                for i in range(DT):
                    nc.sync.dma_start(
                        out=qt[i][:, h * H:(h + 1) * H],
                        in_=QT[i * P:(i + 1) * P, h * H:(h + 1) * H],
                    )
            nc.vector.memset(ones[:], 1.0)

            # ---- per-core K sum-of-squares + AllReduce -> ||K_l||^2 ----
            for h in range(2):
                for i in range(DT):
                    hs = slice(h * H, (h + 1) * H)
                    nc.vector.tensor_mul(sq[i][:, hs], kt[i][:, hs], kt[i][:, hs])
            ss_ps = psum.tile([P, LT], f32, tag="oss")
            for lt in range(LT):
                for dt in range(DT):
                    nc.tensor.matmul(
                        ss_ps[:, lt:lt + 1],
                        sq[dt][:, lt * P:(lt + 1) * P],
                        ones[:],
                        start=(dt == 0),
                        stop=(dt == DT - 1),
                    )
            nc.vector.tensor_copy(ssb[:], ss_ps[:])
            bounce_in = dram.tile([P, LT], f32)
            bounce_out = dram.tile([P, LT], f32)

            def _vload(c):
                vsrc = Vn[c * LC * P:(c + 1) * LC * P, :].rearrange(
                    "(t p) c2 -> p t c2", p=P
                )
                vdst = vall[c][:].rearrange("p (t c2) -> p t c2", c2=CP)
                nc.sync.dma_start(out=vdst, in_=vsrc)

            # The DMA engines grant transfers in strict request-FIFO order
            # and the scheduler issues descriptor gens by readiness, so the
            # big V transfers would be requested before the tiny AllReduce
            # bounce and block it for ~3us.  Tiny "stamp" copies into the
            # first V columns create real data deps that hold each V load
            # back until the AR transfer ahead of it is already requested
            # (the DMA then overwrites the stamp).
            nc.sync.dma_start(bounce_in[:], ssb[:])
            nc.vector.tensor_copy(vall[0][:, 0:2], ssb[:, 0:2])
            _vload(0)
            if _mock_collective:
                nc.gpsimd.dma_start(bounce_out[:], bounce_in[:])
            else:
                nc.gpsimd.collective_compute(
                    "AllReduce",
                    mybir.AluOpType.add,
                    replica_groups=[list(range(B))],
                    ins=[bounce_in.opt()],
                    outs=[bounce_out.opt()],
                )
            nc.sync.dma_start(sstot[:], bounce_out[:])

            # ---- -0.5*sqrt(ss) via Newton rsqrt on DVE (keeps ACT
            # pure-Exp). ss ~ chi^2(2048)*0.02^2 concentrates near 0.82,
            # so a constant seed converges below fp32 eps in 3 steps. ----
            rnw = cst.tile([P, LT], f32, tag="rnw")
            tnw = cst.tile([P, LT], f32, tag="tnw")
            nc.vector.memset(rnw[:], 1.104)
            for it in range(3):
                nc.vector.tensor_mul(tnw[:], rnw[:], rnw[:])
                nc.vector.tensor_mul(tnw[:], sstot[:], tnw[:])
                if it == 0:
                    # V1 stamp: depends on sstot, so V1's DMA request
                    # trails the whole AR chain instead of blocking it
                    nc.vector.tensor_copy(vall[1][:, 0:2], tnw[:, 0:2])
                    _vload(1)
                nc.vector.tensor_scalar(
                    tnw[:], tnw[:], -0.5, 1.5,
                    mybir.AluOpType.mult, mybir.AluOpType.add,
                )
                nc.vector.tensor_mul(rnw[:], rnw[:], tnw[:])
            nc.vector.tensor_mul(nrm[:], sstot[:], rnw[:])
            nc.vector.tensor_scalar_mul(biasn[:], nrm[:], -0.5)

            # ---- phiQ (un-normalized: scale cancels in num/den) ----
            for g in range(L // NQ):
                for mt in range(MT):
                    pq_ps = psum.tile([P, NQ], f32, tag="pq")
                    for dt in range(DT):
                        nc.tensor.matmul(
                            pq_ps[:],
                            pt[dt][:, mt * P:(mt + 1) * P],
                            qt[dt][:, g * NQ:(g + 1) * NQ],
                            start=(dt == 0),
                            stop=(dt == DT - 1),
                        )
                    nc.scalar.activation(
                        eq[mt][:, g * NQ:(g + 1) * NQ], pq_ps[:], AF.Exp,
                    )

            # ---- phiK = exp(K@proj.T - 0.5*nrm) ----
            for lt in range(LT):
                pk_ps = psum.tile([P, D], f32, tag="pk")
                for dt in range(DT):
                    nc.tensor.matmul(
                        pk_ps[:],
                        kt[dt][:, lt * P:(lt + 1) * P],
                        pt[dt][:],
                        start=(dt == 0),
                        stop=(dt == DT - 1),
                    )
                nc.scalar.activation(
                    ek[:, lt * D:(lt + 1) * D], pk_ps[:], AF.Exp,
                    bias=biasn[:, lt:lt + 1],
                )

            # ---- KV state S|z = phiK^T @ [V|1|0] ----
            s_ps = [psums.tile([P, CP], f32, tag=f"s{mt}", name=f"s{mt}")
                    for mt in range(MT)]
            for c in range(NC):
                for j in range(LC):
                    lt = c * LC + j
                    for mt in range(MT):
                        nc.tensor.matmul(
                            s_ps[mt][:],
                            ek[:, lt * D + mt * P: lt * D + mt * P + P],
                            vall[c][:, j * CP:(j + 1) * CP],
                            start=(c == 0 and j == 0),
                            stop=(c == NC - 1 and j == LC - 1),
                        )
            s_sb = []
            for mt in range(MT):
                t = cst.tile([P, CP], f32r, tag=f"sstate{mt}", name=f"sstate{mt}")
                nc.vector.tensor_copy(t[:], s_ps[mt][:])
                s_sb.append(t)

            # ---- all 16 denominators in one go: den[l] = phiQ[l].z via
            # tiny 2-col matmuls (fp32r needs even widths; the V pad col
            # duplicates z so the extra lane is finite), then a single
            # reciprocal ----
            # reuses the s0 bank (freed once s_sb is copied out)
            den_ps = psums.tile([P, 2 * LT], f32, tag="s0")
            for lt in range(LT):
                for mt in range(MT):
                    nc.tensor.matmul(
                        den_ps[:, 2 * lt:2 * lt + 2],
                        eq[mt][:, lt * P:(lt + 1) * P],
                        s_sb[mt][:, D:D + 2],
                        start=(mt == 0),
                        stop=(mt == MT - 1),
                    )
            rdall = cst.tile([P, 2 * LT], f32, tag="rdall")
            nc.vector.reciprocal(rdall[:], den_ps[:])

            # ---- num = phiQ @ [S|z]; out = num * (1/den), the scale ops
            # alternating DVE / ACT so neither engine is the tail ----
            for lt in range(LT):
                o_ps = psum.tile([P, CP], f32, tag="oss")
                for mt in range(MT):
                    nc.tensor.matmul(
                        o_ps[:],
                        eq[mt][:, lt * P:(lt + 1) * P],
                        s_sb[mt][:],
                        start=(mt == 0),
                        stop=(mt == MT - 1),
                    )
                odst_sb = obig[:, lt * D:(lt + 1) * D]
                if lt % 2 == 0:
                    nc.vector.tensor_scalar_mul(
                        odst_sb, o_ps[:, 0:D], rdall[:, 2 * lt:2 * lt + 1]
                    )
                else:
                    nc.scalar.activation(
                        odst_sb, o_ps[:, 0:D], AF.Copy,
                        scale=rdall[:, 2 * lt:2 * lt + 1],
                    )
                if lt % SG == SG - 1:
                    k = lt // SG
                    osrc = obig[:, k * SG * D:(k + 1) * SG * D].rearrange(
                        "p (t c) -> p t c", c=D
                    )
                    odst = OUT[k * SG * P:(k + 1) * SG * P, :].rearrange(
                        "(t p) c -> p t c", p=P
                    )
                    nc.sync.dma_start(out=odst, in_=osrc)

    nc.compile()
    return nc


def _get_nc():
    if "nc" not in _CACHE:
        _CACHE["nc"] = _build()
    return _CACHE["nc"]


def kernel(Q=None, K=None, V=None, sent_embed_slice=None, proj=None,
           qkv_size=None, **extra):
    import ml_dtypes

    bf = ml_dtypes.bfloat16
    Q = np.ascontiguousarray(np.asarray(Q, dtype=np.float32))
    K = np.ascontiguousarray(np.asarray(K, dtype=np.float32))
    V = np.ascontiguousarray(np.asarray(V, dtype=np.float32))
    proj = np.ascontiguousarray(np.asarray(proj, dtype=np.float32))
    PTh = np.ascontiguousarray(proj.T.astype(bf))

    in_maps = []
    for b in range(B):
        vp = np.zeros((L, D + 2), dtype=np.float32)
        vp[:, :D] = V[b]
        vp[:, D] = 1.0
        vp[:, D + 1] = 1.0
        in_maps.append({
            "KT": np.ascontiguousarray(K[b].T.astype(bf)),
            "QT": np.ascontiguousarray(Q[b].T.astype(bf)),
            "V": vp,
            "PT": PTh,
        })

    nc = _get_nc()

    if os.environ.get("BASS_KERNEL_SIM"):
        from concourse import bass_interp

        sim = bass_interp.MultiCoreSim(nc, num_cores=B)
        for i in range(B):
            for k, v in in_maps[i].items():
                sim.cores[i].tensor(k)[:] = v
        sim.simulate(check_with_hw=False)
        out = np.stack(
            [np.array(sim.cores[i].tensor("OUT")) for i in range(B)], axis=0
        )
        return out.astype(np.float32)

    from concourse.bass_utils import run_bass_kernel_spmd

    trace = bool(os.environ.get("BASS_KERNEL_TRACE"))
    tdir = os.environ.get("BASS_KERNEL_TRACE_DIR") or None
    res = run_bass_kernel_spmd(nc, in_maps, list(range(B)), trace=trace,
                               tmpdir=tdir)
    _CACHE["last_result"] = res
    out = np.stack([res.results[i]["OUT"] for i in range(B)], axis=0)
    return out.astype(np.float32)



# revision 4
# speedup vs baseline: 3.0677x; 3.0677x over previous
"""Performer (FAVOR+) linear attention kernel for Trainium2, 8 NeuronCores.

Problem (hardcoded): B=8, L=2048, D=M=256, fp32.
  phi(X)[b,l,m] = exp(X[b,l]@proj[m] - 0.5*||X[:,l,:]||_F) / sqrt(M)
  S = phiK^T V (per batch), z = sum_l phiK, out = (phiQ@S) / (phiQ.z)

Sharding: data-parallel over batch, one batch per core, no collectives.
The norm factors are handled algebraically:
  - phiQ's exp(-0.5*nrm_l) is constant across m, so it cancels in num/den
    (as do both 1/sqrt(M) factors) and is skipped.
  - phiK's factor enters S and z linearly, so w_l = exp(-0.5*||K_l||_F) is
    folded into the host-side prep of V (V'_l = w_l V_l, ones-col -> w).
    The device never needs the cross-batch norm; the 8-core AllReduce that
    used to dominate (launch skew made the 8KB collective ~47us) is gone.

Device pipeline per core (pure bf16 GEMMs, fp32 PSUM accumulate):
  pk = K@proj^T -> exp -> ek          (1024-wide ACT calls, 4 of them)
  pq = proj@Q^T -> exp -> eq          (phiQ^T layout; 1024-wide calls)
  S|z = ek^T @ [V'|w]                 (accumulated over all 16 l-tiles)
  num|den = eq^T @ [S|z]              (den rides along as column 256)
  OUT = [num|den] bf16                (division happens on the host)

Perf notes (from NTFF traces): every dma_start occupies the issuing HWDGE
sequencer ~0.6-0.8us, so inputs load in 7 large DMAs ordered by need (PT,
K halves, then V/Q halves interleaved); num/den go to DRAM unscaled so no
per-tile reciprocal chain serializes the tail; num pairs write even/odd
tiles into the two banks of one 4KB PSUM tile so one strided copy moves
both; PSUM tags: one shared 2-slot 4KB rotation (warmup/pk/pq/num-pairs)
+ 2 banks of persistent S state = 6 of 8 banks. Warmup junk matmuls and a
junk exp at t=0 pull the HAM clock-gate ramp and the ~2.7us exp-table
load into the fixed ~7us NEFF preamble + first DMA wait.
"""

import os
import numpy as np

B = 8
L = 2048
D = 256
P = 128
LT = L // P     # 16 l-tiles of 128
DT = D // P     # 2 d-stripes of 128
MT = D // P     # 2 m-stripes of 128
NQ = 512        # moving free-size (psum-bank limit) for phiQ matmuls
CP = D + 1      # V' | w  /  num | den
GK = 4          # l-tiles per phiK exp group (1024-wide ACT calls)
NGK = LT // GK
SG = 4          # l-tiles per output store

_CACHE = {}


def _build():
    from concourse import bass, bacc, tile

    mybir = bass.mybir
    f32 = mybir.dt.float32
    bf16 = mybir.dt.bfloat16
    AF = mybir.ActivationFunctionType

    nc = bacc.Bacc("TRN2", target_bir_lowering=False, debug=False, num_devices=B)

    KT = nc.declare_dram_parameter("KT", [D, L], bf16, isOutput=False)
    QT = nc.declare_dram_parameter("QT", [D, L], bf16, isOutput=False)
    Vn = nc.declare_dram_parameter("V", [L, CP], bf16, isOutput=False)
    PT = nc.declare_dram_parameter("PT", [D, D], bf16, isOutput=False)
    OUT = nc.declare_dram_parameter("OUT", [L, CP], bf16, isOutput=True)

    with tile.TileContext(nc) as tc:
        with (
            tc.tile_pool(name="cst", bufs=1) as cst,
            tc.tile_pool(name="psum", bufs=2, space="PSUM") as psum,
            tc.tile_pool(name="psums", bufs=1, space="PSUM") as psums,
        ):
            pt = cst.tile([P, 2 * D], bf16, tag="pt")
            kt = cst.tile([P, 2 * L], bf16, tag="kt")
            qt = cst.tile([P, 2 * L], bf16, tag="qt")
            vall = cst.tile([P, LT * CP], bf16, tag="vall")
            ek = cst.tile([P, LT * D], bf16, tag="ek")
            eq = [cst.tile([P, L], bf16, tag=f"eq{i}", name=f"eq{i}")
                  for i in range(MT)]
            obig = cst.tile([P, LT * CP], bf16, tag="obig")
            junk = cst.tile([P, D], bf16, tag="junk")
            jexp = cst.tile([P, D], bf16, tag="jexp")

            # ---- warmups, zero input deps: spin the PE so the HAM clock
            # gate flips to 8/8 inside the preamble/DMA window, and fire a
            # junk exp so ACT's exp-table load overlaps the loads too ----
            nc.vector.memset(junk[:], 0.5)
            jps = psum.tile([P, D], f32, tag="big")
            NW = 8
            for w in range(NW):
                nc.tensor.matmul(jps[:], junk[:, 0:P], junk[:],
                                 start=(w == 0), stop=(w == NW - 1))
            nc.scalar.activation(jexp[:], junk[:], AF.Exp)

            # ---- input loads: 7 large DMAs on the SP HWDGE ring, in
            # need-order (each dma_start costs ~0.6-0.8us of sequencer
            # time and streams share HBM, so order == priority) ----
            def _kq2(dst, src, h):
                H = L // 2
                s3 = src.rearrange("(dt p) l -> p dt l", p=P)
                d3 = dst[:].rearrange("p (dt l) -> p dt l", l=L)
                nc.sync.dma_start(
                    out=d3[:, :, h * H:(h + 1) * H],
                    in_=s3[:, :, h * H:(h + 1) * H],
                )

            def _vload(c):
                HL = LT // 2
                vsrc = Vn[c * HL * P:(c + 1) * HL * P, :].rearrange(
                    "(t p) c2 -> p t c2", p=P
                )
                vdst = vall[:, c * HL * CP:(c + 1) * HL * CP].rearrange(
                    "p (t c2) -> p t c2", c2=CP
                )
                nc.sync.dma_start(out=vdst, in_=vsrc)

            nc.sync.dma_start(
                out=pt[:].rearrange("p (i m) -> p i m", m=D),
                in_=PT.rearrange("(i p) m -> p i m", p=P),
            )
            _kq2(kt, KT, 0)
            _kq2(kt, KT, 1)
            _vload(0)
            _kq2(qt, QT, 0)
            _vload(1)
            _kq2(qt, QT, 1)

            # ---- phiK = exp(K@proj^T), 4 l-tiles per 1024-wide exp ----
            for g in range(NGK):
                pk_ps = psum.tile([P, GK * D], f32, tag="big")
                for j in range(GK):
                    lt = g * GK + j
                    for dt in range(DT):
                        nc.tensor.matmul(
                            pk_ps[:, j * D:(j + 1) * D],
                            kt[:, dt * L + lt * P: dt * L + (lt + 1) * P],
                            pt[:, dt * D:(dt + 1) * D],
                            start=(dt == 0),
                            stop=(dt == DT - 1),
                        )
                nc.scalar.activation(
                    ek[:, g * GK * D:(g + 1) * GK * D], pk_ps[:], AF.Exp,
                )

            # ---- S|z = phiK^T @ [V'|w] for the first l-half, then phiQ
            # for the first q-half, then the second halves (matches DMA
            # arrival order; the Tile scheduler dataflows around gaps) ----
            s_ps = [psums.tile([P, CP], f32, tag=f"s{mt}", name=f"s{mt}")
                    for mt in range(MT)]

            def _s_half(h):
                for lt in range(h * (LT // 2), (h + 1) * (LT // 2)):
                    for mt in range(MT):
                        nc.tensor.matmul(
                            s_ps[mt][:],
                            ek[:, lt * D + mt * P: lt * D + mt * P + P],
                            vall[:, lt * CP:(lt + 1) * CP],
                            start=(lt == 0),
                            stop=(lt == LT - 1),
                        )

            def _pq_half(c):
                for mt in range(MT):
                    pq_ps = psum.tile([P, 2 * NQ], f32, tag="big")
                    for g2 in range(2):
                        for dt in range(DT):
                            nc.tensor.matmul(
                                pq_ps[:, g2 * NQ:(g2 + 1) * NQ],
                                pt[:, dt * D + mt * P: dt * D + mt * P + P],
                                qt[:, dt * L + c * 2 * NQ + g2 * NQ:
                                      dt * L + c * 2 * NQ + (g2 + 1) * NQ],
                                start=(dt == 0),
                                stop=(dt == DT - 1),
                            )
                    nc.scalar.activation(
                        eq[mt][:, c * 2 * NQ:(c + 1) * 2 * NQ], pq_ps[:],
                        AF.Exp,
                    )

            _s_half(0)
            _pq_half(0)
            _s_half(1)
            _pq_half(1)

            s_sb = []
            for mt in range(MT):
                t = cst.tile([P, CP], bf16, tag=f"sstate{mt}", name=f"sstate{mt}")
                nc.vector.tensor_copy(t[:], s_ps[mt][:])
                s_sb.append(t)

            # ---- num|den = phiQ @ [S|z], two l-tiles per 4KB psum tile
            # (even tile -> bank A cols 0:257, odd tile -> bank B cols
            # 512:769) so ONE strided copy drains both; copies alternate
            # DVE / ACT so neither engine is the tail; no division on
            # device (host divides num by den) ----
            for pr in range(LT // 2):
                o_ps = psum.tile([P, 2 * NQ], f32, tag="big")
                for half in range(2):
                    lt = 2 * pr + half
                    for mt in range(MT):
                        nc.tensor.matmul(
                            o_ps[:, half * NQ: half * NQ + CP],
                            eq[mt][:, lt * P:(lt + 1) * P],
                            s_sb[mt][:],
                            start=(mt == 0),
                            stop=(mt == MT - 1),
                        )
                osrc = o_ps[:].rearrange("p (two c) -> p two c", c=NQ)[:, :, 0:CP]
                odst = obig[:, 2 * pr * CP:(2 * pr + 2) * CP].rearrange(
                    "p (two c) -> p two c", c=CP
                )
                if pr % 2 == 0:
                    nc.vector.tensor_copy(odst, osrc)
                else:
                    nc.scalar.activation(odst, osrc, AF.Copy)
                if pr % 2 == 1:
                    k = pr // 2
                    osrc2 = obig[:, k * SG * CP:(k + 1) * SG * CP].rearrange(
                        "p (t c) -> p t c", c=CP
                    )
                    odst2 = OUT[k * SG * P:(k + 1) * SG * P, :].rearrange(
                        "(t p) c -> p t c", p=P
                    )
                    nc.sync.dma_start(out=odst2, in_=osrc2)

    nc.compile()
    return nc


def _get_nc():
    if "nc" not in _CACHE:
        _CACHE["nc"] = _build()
    return _CACHE["nc"]


def kernel(Q=None, K=None, V=None, sent_embed_slice=None, proj=None,
           qkv_size=None, **extra):
    import ml_dtypes

    bf = ml_dtypes.bfloat16
    Q = np.ascontiguousarray(np.asarray(Q, dtype=np.float32))
    K = np.ascontiguousarray(np.asarray(K, dtype=np.float32))
    V = np.ascontiguousarray(np.asarray(V, dtype=np.float32))
    proj = np.ascontiguousarray(np.asarray(proj, dtype=np.float32))
    PTh = np.ascontiguousarray(proj.T.astype(bf))

    # per-timestep Frobenius norm over ALL batches, folded into V on the
    # host (exact; frees the device of the cross-batch AllReduce)
    nrm = np.sqrt(np.sum(K.astype(np.float64) ** 2, axis=(0, 2)))
    w = np.exp(-0.5 * nrm).astype(np.float32)       # (L,)

    in_maps = []
    for b in range(B):
        vp = np.empty((L, CP), dtype=np.float32)
        vp[:, :D] = V[b] * w[:, None]
        vp[:, D] = w
        in_maps.append({
            "KT": np.ascontiguousarray(K[b].T.astype(bf)),
            "QT": np.ascontiguousarray(Q[b].T.astype(bf)),
            "V": vp.astype(bf),
            "PT": PTh,
        })

    nc = _get_nc()

    def _finish(raw):
        nd = raw.astype(np.float32)
        return nd[:, :D] / nd[:, D:D + 1]

    if os.environ.get("BASS_KERNEL_SIM"):
        from concourse import bass_interp

        nsim = int(os.environ.get("BASS_KERNEL_SIM_CORES") or B)
        sim = bass_interp.MultiCoreSim(nc, num_cores=nsim)
        for i in range(nsim):
            for k, v in in_maps[i].items():
                sim.cores[i].tensor(k)[:] = v
        sim.simulate(check_with_hw=False)
        out = np.stack(
            [_finish(np.array(sim.cores[i].tensor("OUT"))) for i in range(nsim)]
            + [np.zeros((L, D), dtype=np.float32)] * (B - nsim),
            axis=0,
        )
        return out.astype(np.float32)

    from concourse.bass_utils import run_bass_kernel_spmd

    trace = bool(os.environ.get("BASS_KERNEL_TRACE"))
    tdir = os.environ.get("BASS_KERNEL_TRACE_DIR") or None
    res = run_bass_kernel_spmd(nc, in_maps, list(range(B)), trace=trace,
                               tmpdir=tdir)
    _CACHE["last_result"] = res
    out = np.stack([_finish(res.results[i]["OUT"]) for i in range(B)], axis=0)
    return out.astype(np.float32)
